# revision 1
# baseline (speedup 1.0000x reference)
"""Trainium2 Bass kernel for a 3x3 'same' conv: x [8,16,512,512] f32, weight [16,144].

Data-parallel over batch: 1 image per NeuronCore, 8 cores.

Measured facts on this machine that drive the design:
  - PE cost is per streamed matmul column (LDWEIGHTS+512-col matmul ~= 350 ns
    warm), so minimize passes: 3 per group (one per kw tap -- each pass has a
    fixed horizontal shift), with row slots for all (r, kh) combos needing
    16*(R+2) <= 128 partitions => R=6 output rows per group, 86 groups,
    258 matmuls ~= 90 us PE span. fp32r matmuls stream at half clock -- use
    fp16 operands (fp32 PSUM accumulation) instead.
  - DMA time scales with bytes moved (~14-23 GB/s per SDMA engine, 16
    engines); fp16 x + fp16 out nearly halves bytes vs fp32 at ~1e-4 cost.

Modes (CONV_MODE env):
  f16  (default): x/weights fp16, fp32 PSUM accumulate + fp32 output.
  f16o: fp16 output too (host upcasts); fewest bytes, adds ~2.4e-4 rounding.
  f32r: all-fp32 (relaxed-precision fp32r matmul); most accurate (~1.5e-4).

Structure (all modes):
  - Host pads x columns to [16, 512, 514] with zero cols 0 and 513 so the
    horizontal taps become plain SBUF column offsets (no device memsets,
    full-bank PSUM writes -- fp32r's dst-pattern ISA restriction).
  - Group g covers output rows [y0, y0+6); its x-tile holds the 8-row padded
    window at partition p = ci*8 + j (row Y+j, Y = clamp(y0-1, 0, 504)),
    K = 128. Three accumulating matmuls (kw = 0,1,2; rhs columns [kw, kw+512))
    into one PSUM bank [96, 512] (M = 16 co x 6 rows).
  - Stationary weights per (kw, boundary variant b): [128, 96] matrices
    wk[ci*8+j, co*6+r] = w[co, ci, j-r-(b-1), kw]; entries whose target row
    falls outside the window are dropped (those are the zero-pad rows).
  - PSUM -> SBUF via VectorE copy; input DMAs on the sync HWDGE queue,
    output DMAs on the scalar HWDGE queue.
"""

import os
from contextlib import ExitStack

import numpy as np

C_OUT, C_IN, KH, KW = 16, 16, 3, 3
H = W = 512
WP = W + 2      # host-padded row length
B = 8
R = 6           # output rows per group
J = R + 2      # input rows per group
M = C_OUT * R   # 96 psum partitions
K = C_IN * J    # 128 contraction partitions
NV = KW * 3     # stationary variants: kw x boundary
GROUP_Y0 = [6 * g for g in range(85)] + [506]

MODE = os.environ.get("CONV_MODE", "f16o")  # f16o | f16 | f32r

_CACHE = {}


def _build_weights(weight: np.ndarray) -> np.ndarray:
    """[16,144] -> [128, 9*96] stationary matrices, variant v = kw*3 + b.

    wk[ci*J+j, v, co*R+r] = w[co, ci, kh, kw] at j = r + kh + (b-1); (r, kh)
    with j outside [0, J) dropped (they reference the zero-pad rows).
    """
    w = np.asarray(weight, dtype=np.float32).reshape(C_OUT, C_IN, KH, KW)
    wk = np.zeros((KW, 3, K, M), np.float32)
    for kw in range(KW):
        for b in range(3):
            for co in range(C_OUT):
                for r in range(R):
                    for kh in range(KH):
                        j = r + kh + (b - 1)
                        if 0 <= j < J:
                            for ci in range(C_IN):
                                wk[kw, b, ci * J + j, co * R + r] = w[co, ci, kh, kw]
    out = np.ascontiguousarray(wk.transpose(2, 0, 1, 3).reshape(K, NV * M))
    return out if MODE == "f32r" else out.astype(np.float16)


def _build_nc():
    import concourse.tile as tile
    from concourse import bacc, mybir

    f32 = mybir.dt.float32
    dt_in = mybir.dt.float32r if MODE == "f32r" else mybir.dt.float16
    dt_out = mybir.dt.float16 if MODE == "f16o" else f32

    nc = bacc.Bacc("TRN2", target_bir_lowering=False, debug=False,
                   enable_asserts=False, num_devices=B)
    # for f32r, declaring inputs as the matmul dtype keeps the BIR fp32r
    # producer->consumer chain consistent (same 4-byte layout as float32)
    x = nc.dram_tensor("x", [C_IN, H, WP], dt_in, kind="ExternalInput").ap()
    wkin = nc.dram_tensor("wk", [K, NV * M], dt_in, kind="ExternalInput").ap()
    out = nc.dram_tensor("out", [C_OUT, H, W], dt_out, kind="ExternalOutput").ap()

    with tile.TileContext(nc) as tc, ExitStack() as ctx:
        wpool = ctx.enter_context(tc.tile_pool(name="wpool", bufs=1))
        xpool = ctx.enter_context(tc.tile_pool(name="xpool", bufs=12))
        opool = ctx.enter_context(tc.tile_pool(name="opool", bufs=10))
        ppool = ctx.enter_context(tc.tile_pool(name="ppool", bufs=8, space="PSUM"))

        wt = wpool.tile([K, NV * M], dt_in, name="wt")
        # per-variant weight loads so the first group's stationaries land
        # quickly (first matmul needs only v=0)
        for v in range(NV):
            eng = nc.scalar if v % 2 == 0 else nc.sync
            eng.dma_start(out=wt[:, v * M : (v + 1) * M],
                          in_=wkin[:, v * M : (v + 1) * M])

        # batches of 4 groups, matmuls issued kw-major (4 consecutive
        # matmuls share one stationary)
        GB = 4
        for i0 in range(0, len(GROUP_Y0), GB):
            batch = GROUP_Y0[i0 : i0 + GB]
            bs = [0 if y0 == 0 else (2 if y0 == H - R else 1) for y0 in batch]

            xts, pts = [], []
            for y0 in batch:
                Y = min(max(y0 - 1, 0), H - J)
                xtile = xpool.tile([K, WP], dt_in, name="xtile", tag="xtile")
                nc.sync.dma_start(out=xtile[:], in_=x[:, Y : Y + J, :])
                xts.append(xtile)
                pts.append(ppool.tile([M, W], f32, name="pt", tag="pt"))

            for kw in range(KW):
                for xtile, pt, b in zip(xts, pts, bs):
                    v = kw * 3 + b
                    nc.tensor.matmul(pt[:, 0:W], wt[:, v * M : (v + 1) * M],
                                     xtile[:, kw : kw + W],
                                     start=(kw == 0), stop=(kw == KW - 1))

            for y0, pt in zip(batch, pts):
                ot = opool.tile([M, W], dt_out, name="ot", tag="ot")
                nc.vector.tensor_copy(ot[:], pt[:])
                nc.scalar.dma_start(out=out[:, y0 : y0 + R, :], in_=ot[:])

    nc.compile()
    return nc


def get_nc():
    if MODE not in _CACHE:
        _CACHE[MODE] = _build_nc()
    return _CACHE[MODE]


def run(x: np.ndarray, weight: np.ndarray, **spmd_kwargs):
    """Run the conv on 8 cores; returns (out [8,16,512,512] f32, results)."""
    from concourse.bass_utils import run_bass_kernel_spmd

    x = np.asarray(x, dtype=np.float32)
    np_in = np.float32 if MODE == "f32r" else np.float16
    xp = np.zeros((B, C_IN, H, WP), np_in)
    xp[:, :, :, 1 : W + 1] = x.astype(np_in)
    wk = _build_weights(weight)
    nc = get_nc()
    in_maps = [{"x": xp[b], "wk": wk} for b in range(B)]
    res = run_bass_kernel_spmd(nc, in_maps, list(range(B)), **spmd_kwargs)
    out = np.stack([res.results[b]["out"] for b in range(B)], axis=0)
    if out.dtype != np.float32:
        out = out.astype(np.float32)
    return out, res


def kernel(x: np.ndarray, weight: np.ndarray) -> np.ndarray:
    return run(x, weight)[0]



# revision 3
# speedup vs baseline: 1.3569x; 1.3569x over previous
"""Trainium2 Bass kernel for a 3x3 'same' conv: x [8,16,512,512] f32, weight [16,144].

Data-parallel over batch: 1 image per NeuronCore, 8 cores.

Design (v2):
  - Block-Toeplitz matmul scheme: group of R=6 output rows; x-window of J=8
    rows lives at partition (ci*8 + j), K=128; three accumulating matmuls
    (kw = 0,1,2, moving slice shifted by kw) into one PSUM bank.
  - Stationary padded to M=128 columns (96 used: co*6+r) so the compiler's
    Fast Weight Load path (NumWeights==128) kicks in.
  - LDWEIGHTS dedup: tile_legalize is wrapped to drop InstLdweights that
    reload the stationary already in the PE array (kw-major issue order
    makes runs of 4+ matmuls share one stationary). ~258 -> ~60 loads.
  - Host prepares a partition-major input layout xh[128, 86, 514] (window
    rows duplicated) so each group-batch input DMA has 4KB-contiguous
    descriptors; output goes to a partition-major od[96, 86, 512] fp16
    staging tensor (4KB descriptors) and the host scatters/upcasts.
  - fp16 operands and fp16 staged output (error ~5e-4 vs fp32 reference).
  - PSUM->SBUF casts alternate Vector/Scalar engines so PSUM recycling
    keeps up with the PE.
"""

import os
from contextlib import ExitStack

import numpy as np

C_OUT, C_IN, KH, KW = 16, 16, 3, 3
H = W = 512
WP = W + 2      # padded row length (zero col 0 and 513)
B = 8
R = 6           # output rows per group
J = R + 2       # input window rows per group
M = 128         # stationary columns (co*6+r in [0,96), rest zero-padded)
MU = C_OUT * R  # 96 useful psum partitions
K = C_IN * J    # 128 contraction partitions
NV = KW * 3     # stationary variants: kw x boundary
NG = 86         # groups: y0 = 6g for g<85, 506 for g=85
GB = 4          # groups per batch
GROUP_Y0 = [6 * g for g in range(85)] + [506]

_CACHE = {}


def _install_ldw_dedup():
    """Wrap tile_legalize with a pass that removes InstLdweights which
    reload the stationary already loaded in the PE array (same weights AP,
    only non-transpose matmuls / non-PE instructions in between)."""
    import concourse.tile as tilemod
    from concourse import mybir

    if getattr(tilemod, "_ldw_dedup_installed", False):
        return
    orig = tilemod.tile_legalize
    PE = mybir.EngineType.PE

    def _sig(i):
        tp = i.tile_position
        return (str(i.ins[0]), str(i.perf_mode), bool(i.is_transpose),
                None if tp is None else tuple(tp))

    def dedup(ordered, nc):
        out = orig(ordered, nc)
        for bb in list(out.keys()):
            cur = None
            keep = []
            for i in out[bb]:
                if isinstance(i, mybir.InstLdweights):
                    s = _sig(i)
                    if cur is not None and cur == s:
                        continue
                    cur = s
                elif isinstance(i, mybir.InstMatmult):
                    if i.is_transpose:
                        cur = None
                elif i.engine == PE and type(i).__name__ not in (
                        "InstEventSemaphore", "InstNotify", "InstNop"):
                    cur = None
                keep.append(i)
            out[bb] = keep
        return out

    tilemod.tile_legalize = dedup
    tilemod._ldw_dedup_installed = True


def _ystart(g):
    return min(max(GROUP_Y0[g] - 1, 0), H - J)


def _bvar(g):
    if g == 0:
        return 0
    if g == NG - 1:
        return 2
    return 1


def _build_weights(weight: np.ndarray) -> np.ndarray:
    """[16,144] -> [128, 9*128] stationary matrices, variant v = kw*3 + b.

    wk[ci*J+j, v*128 + co*R+r] = w[co, ci, kh, kw] at j = r + kh + (b-1);
    (r, kh) with j outside [0, J) dropped (zero-pad rows). Columns 96..127
    of each variant are zero (pad to 128 for fast weight load)."""
    w = np.asarray(weight, dtype=np.float32).reshape(C_OUT, C_IN, KH, KW)
    wk = np.zeros((KW, 3, K, M), np.float32)
    for kw in range(KW):
        for b in range(3):
            for co in range(C_OUT):
                for r in range(R):
                    for kh in range(KH):
                        j = r + kh + (b - 1)
                        if 0 <= j < J:
                            for ci in range(C_IN):
                                wk[kw, b, ci * J + j, co * R + r] = w[co, ci, kh, kw]
    out = np.ascontiguousarray(wk.transpose(2, 0, 1, 3).reshape(K, NV * M))
    return out.astype(np.float16)


def _prep_x(x: np.ndarray) -> np.ndarray:
    """[8,16,512,512] f32 -> xh [8, 128, 86, 514] fp16, partition-major:
    xh[b, ci*8+j, g, :] = zero-padded row (Ystart(g)+j) of image b/ci."""
    xp = np.zeros((B, C_IN, H, WP), np.float16)
    xp[:, :, :, 1:W + 1] = x.astype(np.float16)
    rows = np.empty((NG, J), np.int64)
    for g in range(NG):
        rows[g] = _ystart(g) + np.arange(J)
    # [B, 16, 86, 8, 514] -> [B, 16, 8, 86, 514] -> [B, 128, 86, 514]
    xh = xp[:, :, rows, :].transpose(0, 1, 3, 2, 4)
    return np.ascontiguousarray(xh.reshape(B, K, NG, WP))


def _unpack_out(od: np.ndarray) -> np.ndarray:
    """od [8, 96, 86, 512] fp16 -> [8, 16, 512, 512] f32."""
    blk = od.reshape(B, C_OUT, R, NG, W).transpose(0, 1, 3, 2, 4)
    out = np.empty((B, C_OUT, H, W), np.float32)
    out[:, :, :510] = blk[:, :, :85].reshape(B, C_OUT, 510, W)
    out[:, :, 506:512] = blk[:, :, 85]
    return out


def _build_nc():
    import concourse.tile as tile
    from concourse import bacc, mybir

    if os.environ.get("CONV_NO_DEDUP", "0") != "1":
        _install_ldw_dedup()

    f32 = mybir.dt.float32
    f16 = mybir.dt.float16

    nc = bacc.Bacc("TRN2", target_bir_lowering=False, debug=False,
                   enable_asserts=False, num_devices=B)
    xh = nc.dram_tensor("xh", [K, NG, WP], f16, kind="ExternalInput").ap()
    wkin = nc.dram_tensor("wk", [K, NV * M], f16, kind="ExternalInput").ap()
    od = nc.dram_tensor("od", [MU, NG, W], f16, kind="ExternalOutput").ap()

    batches = [list(range(i, min(i + GB, NG))) for i in range(0, NG, GB)]

    with tile.TileContext(nc) as tc, ExitStack() as ctx:
        wpool = ctx.enter_context(tc.tile_pool(name="wpool", bufs=1))
        xpool = ctx.enter_context(tc.tile_pool(name="xpool", bufs=3))
        opool = ctx.enter_context(tc.tile_pool(name="opool", bufs=3))
        ppool = ctx.enter_context(tc.tile_pool(name="ppool", bufs=8, space="PSUM"))

        wt = wpool.tile([K, NV * M], f16, name="wt")
        # per-variant loads so the first batch's stationaries land quickly
        for v in range(NV):
            eng = nc.scalar if v % 2 == 0 else nc.sync
            eng.dma_start(out=wt[:, v * M:(v + 1) * M],
                          in_=wkin[:, v * M:(v + 1) * M])

        for bi, batch in enumerate(batches):
            nb = len(batch)
            g0 = batch[0]
            xtile = xpool.tile([K, nb * WP], f16, name="xtile", tag="xtile")
            if bi == 0:
                # split so the first matmul waits only on group 0's slab
                for i in range(nb):
                    nc.sync.dma_start(out=xtile[:, i * WP:(i + 1) * WP],
                                      in_=xh[:, g0 + i, :])
            else:
                nc.sync.dma_start(out=xtile[:], in_=xh[:, g0:g0 + nb, :])

            pts = [ppool.tile([M, W], f32, name="pt", tag="pt") for _ in batch]

            kws = range(KW) if bi % 2 == 0 else range(KW - 1, -1, -1)
            for ki, kw in enumerate(kws):
                for i, g in enumerate(batch):
                    v = kw * 3 + _bvar(g)
                    nc.tensor.matmul(pts[i][:, 0:W],
                                     wt[:, v * M:(v + 1) * M],
                                     xtile[:, i * WP + kw: i * WP + kw + W],
                                     start=(ki == 0), stop=(ki == KW - 1))

            ot = opool.tile([MU, nb * W], f16, name="ot", tag="ot")
            for i, g in enumerate(batch):
                if i % 2 == 0:
                    nc.vector.tensor_copy(ot[:, i * W:(i + 1) * W], pts[i][0:MU, :])
                else:
                    nc.scalar.copy(ot[:, i * W:(i + 1) * W], pts[i][0:MU, :])
            nc.scalar.dma_start(out=od[:, g0:g0 + nb, :], in_=ot[:])

    nc.compile()
    return nc


def get_nc():
    if "nc" not in _CACHE:
        _CACHE["nc"] = _build_nc()
    return _CACHE["nc"]


def run(x: np.ndarray, weight: np.ndarray, **spmd_kwargs):
    """Run the conv on 8 cores; returns (out [8,16,512,512] f32, results)."""
    from concourse.bass_utils import run_bass_kernel_spmd

    x = np.asarray(x, dtype=np.float32)
    xh = _prep_x(x)
    wk = _build_weights(weight)
    nc = get_nc()
    in_maps = [{"xh": xh[b], "wk": wk} for b in range(B)]
    res = run_bass_kernel_spmd(nc, in_maps, list(range(B)), **spmd_kwargs)
    od = np.stack([res.results[b]["od"] for b in range(B)], axis=0)
    return _unpack_out(od), res


def kernel(x: np.ndarray, weight: np.ndarray) -> np.ndarray:
    return run(x, weight)[0]
